# revision 20
# baseline (speedup 1.0000x reference)
"""EuclideanCodebook (VQ) Trainium2 kernel.

Data-parallel over 8 NeuronCores: core i handles batch row i (8192 tokens,
64 tiles of 128). Per 128-token tile:
  score = x @ embed.T - 0.5*|e|^2   via bf16 3-term split matmuls
          (xh*eh + xh*el + xl*eh, fp32 PSUM accumulate; x split on-chip
          after a PE transpose, embed split on host) + a 3-row ones
          matmul folding -0.5*|e|^2, ~4e-4 abs accuracy -> zero argmax
          flips vs the fp32 reference.
  dist  = -2*score + |x|^2          (ACT Identity, per-partition bias)
  ind   = argmax(score)             (DVE max/max_index; ties -> first)
  quant = embed[ind]                (SWDGE indirect-DMA gather from HBM)
Tiles are processed in pairs (shared PE transpose + bf16 split,
software-pipelined one pair ahead so the ACT/DVE split never stalls the
matmul stream) and grouped into 8-tile supertiles for 1 MB-granularity
DMA; dist is written back per pair, quant per 4 tiles, indices in two
4 KB chunks. The ones/negee matmul operands span all 128 partitions
(rows 3..127 zero) so their LDWEIGHTS can pull ahead into the background
weight buffer instead of serializing on row-group 0.
Measured ~242 us/core on trn2 (PE-bound: product matmuls issue at the
216 ns/512-col hardware floor, ~84% PE occupancy; memory roofline for
the 49 MB/core of traffic is ~140 us).
"""
import sys
sys.path.insert(0, "/opt/trn_rl_repo")
from contextlib import ExitStack

import numpy as np
import ml_dtypes

import concourse.bass as bass
import concourse.bacc as bacc
import concourse.tile as tile
from concourse import mybir
from concourse.bass_utils import run_bass_kernel_spmd
from concourse.masks import make_identity

F32 = mybir.dt.float32
BF16 = mybir.dt.bfloat16
U32 = mybir.dt.uint32
ACT_T = mybir.ActivationFunctionType
BF = ml_dtypes.bfloat16

B, S, D, K = 8, 8192, 256, 1024
NCORES = 8
ST = 8                      # tiles per supertile
NS = S // (128 * ST)        # supertiles per core (8)
NT = ST * NS                # 64 token tiles per core


def _bf16(a):
    return np.asarray(a, np.float32).astype(BF).astype(np.float32)


def _build_program():
    nc = bacc.Bacc()
    x_d = nc.declare_dram_parameter("x", [S, D], F32, isOutput=False)
    wpack_d = nc.declare_dram_parameter("wpack", [128, 5 * K + 128], BF16, isOutput=False)
    identf_d = nc.declare_dram_parameter("identf", [128, 128], F32, isOutput=False)
    emb_d = nc.declare_dram_parameter("emb", [K, D], F32, isOutput=False)

    dist_d = nc.declare_dram_parameter("dist", [S, K], F32, isOutput=True)
    quant_d = nc.declare_dram_parameter("quant", [S, D], F32, isOutput=True)
    ind_d = nc.declare_dram_parameter("ind", [S], U32, isOutput=True)

    with tile.TileContext(nc) as tc, ExitStack() as ctx:
        sb = ctx.enter_context(tc.tile_pool(name="sb", bufs=1))
        sb2 = ctx.enter_context(tc.tile_pool(name="sb2", bufs=4))
        slab = ctx.enter_context(tc.tile_pool(name="slab", bufs=2))
        slabd = ctx.enter_context(tc.tile_pool(name="slabd", bufs=3))
        xslab = ctx.enter_context(tc.tile_pool(name="xslab", bufs=3))
        ps = ctx.enter_context(tc.tile_pool(name="ps", bufs=2, space="PSUM"))
        psd = ctx.enter_context(tc.tile_pool(name="psd", bufs=3, space="PSUM"))

        identf = sb.tile([128, 128], F32, tag="identf")
        nc.sync.dma_start(identf[:], identf_d[:])
        x_first = xslab.tile([128, ST, D], F32, tag="x")
        nc.sync.dma_start(
            x_first[:, 0:2, :], x_d[0:2 * 128, :].rearrange("(q p) d -> p q d", p=128))
        wp = sb.tile([128, 5 * K + 128], BF16, tag="wpack")
        nc.sync.dma_start(wp[:], wpack_d[:])
        nc.sync.dma_start(
            x_first[:, 2:ST, :],
            x_d[2 * 128:ST * 128, :].rearrange("(q p) d -> p q d", p=128))
        ones3 = wp[:, 5 * K:5 * K + 128]  # rows 3..127 are zero
        ind_cont = sb.tile([128, NT], U32, tag="indc")
        ind8_slab = sb.tile([128, NT * 8], U32, tag="ind8")
        indT = sb.tile([64, 128], U32, tag="indT")

        def ehT(c, ksl):
            return wp[:, c * K:(c + 1) * K][:, ksl]

        def elT(c, ksl):
            return wp[:, 2 * K + c * K:2 * K + (c + 1) * K][:, ksl]

        def negee(ksl):
            return wp[:, 4 * K:5 * K][:, ksl]  # rows 3..127 are zero

        def emit_ind_chunk(half):
            # tiles [half*32, half*32+32) -> ind_d[half*4096 : half*4096+4096]
            csl = slice(half * 32, half * 32 + 32)
            nc.vector.tensor_copy(
                ind_cont[:, csl],
                ind8_slab[:, half * 256:(half + 1) * 256].rearrange(
                    "p (t e) -> p t e", e=8)[:, :, 0])
            for bi in range(4):
                nc.vector.transpose(
                    indT[csl, bi * 32:(bi + 1) * 32],
                    ind_cont[bi * 32:(bi + 1) * 32, csl])
            nc.sync.dma_start(
                ind_d[half * 4096:(half + 1) * 4096].rearrange("(t p) -> t p", p=128),
                indT[csl, :])

        for s in range(NS):
            if s == 0:
                x_sb = x_first
            else:
                x_sb = xslab.tile([128, ST, D], F32, tag="x")
                nc.sync.dma_start(
                    x_sb[:],
                    x_d[s * ST * 128:(s + 1) * ST * 128, :].rearrange(
                        "(q p) d -> p q d", p=128))
            dist_slab = slabd.tile([128, ST, K], F32, tag="dist")
            quant_slab = slab.tile([128, ST, D], F32, tag="quant")

            def split_pair(q0):
                pt = ps.tile([128, 2 * D], F32, tag="pt")
                for j in range(2):
                    for c in range(2):
                        nc.tensor.transpose(
                            pt[:, j * D + c * 128:j * D + (c + 1) * 128],
                            x_sb[:, q0 + j, c * 128:(c + 1) * 128], identf[:])
                xh = sb2.tile([128, 2 * D], BF16, tag="xh")
                nc.scalar.activation(xh[:], pt[:], ACT_T.Copy)
                xl = sb2.tile([128, 2 * D], BF16, tag="xl")
                nc.vector.tensor_sub(xl[:], pt[:], xh[:])
                return xh, xl

            cur = split_pair(0)
            for qp in range(ST // 2):
                q0 = qp * 2
                xh, xl = cur
                if qp < ST // 2 - 1:
                    cur = split_pair(q0 + 2)

                for j in range(2):
                    q = q0 + j
                    t = s * ST + q
                    xsq = sb2.tile([128, 1], F32, tag="xsq")
                    sq_scr = sb2.tile([128, D], F32, tag="sqscr")
                    nc.scalar.activation(sq_scr[:], x_sb[:, q, :], ACT_T.Square,
                                         accum_out=xsq[:])

                    xhj = xh[:, j * D:(j + 1) * D]
                    xlj = xl[:, j * D:(j + 1) * D]
                    pd = psd.tile([128, K], F32, tag="pd")
                    for h in range(2):
                        ksl = slice(h * 512, (h + 1) * 512)
                        mms = [(ones3, negee(ksl)),
                               (xhj[:, 0:128], ehT(0, ksl)), (xhj[:, 128:256], ehT(1, ksl)),
                               (xhj[:, 0:128], elT(0, ksl)), (xhj[:, 128:256], elT(1, ksl)),
                               (xlj[:, 0:128], ehT(0, ksl)), (xlj[:, 128:256], ehT(1, ksl))]
                        for i, (lhs, rhs) in enumerate(mms):
                            nc.tensor.matmul(pd[:, h * 512:(h + 1) * 512], lhs, rhs,
                                             start=(i == 0), stop=(i == len(mms) - 1))

                    nc.scalar.activation(dist_slab[:, q, :], pd[:], ACT_T.Identity,
                                         bias=xsq[:], scale=-2.0)

                    max8 = sb2.tile([128, 8], F32, tag="max8")
                    nc.vector.max(max8[:], pd[:])
                    nc.vector.max_index(ind8_slab[:, t * 8:(t + 1) * 8], max8[:], pd[:])

                    nc.gpsimd.indirect_dma_start(
                        out=quant_slab[:, q, :], out_offset=None, in_=emb_d[:],
                        in_offset=bass.IndirectOffsetOnAxis(
                            ap=ind8_slab[:, t * 8:t * 8 + 1], axis=0))

                # dist out per pair (1 MB)
                base = s * ST * 128
                nc.sync.dma_start(
                    dist_d[base + q0 * 128:base + (q0 + 2) * 128, :].rearrange(
                        "(w p) k -> p w k", p=128),
                    dist_slab[:, q0:q0 + 2, :])
                if qp % 2 == 1:
                    h0 = q0 - 2
                    nc.sync.dma_start(
                        quant_d[base + h0 * 128:base + (q0 + 2) * 128, :].rearrange(
                            "(w p) d -> p w d", p=128),
                        quant_slab[:, h0:q0 + 2, :])
            if s == NS // 2 - 1:
                emit_ind_chunk(0)
        emit_ind_chunk(1)

    nc.finalize()
    return nc


_CACHE = {}


def _prep_in_maps(x, embed):
    x = np.ascontiguousarray(np.asarray(x, dtype=np.float32))
    embed = np.ascontiguousarray(np.asarray(embed, dtype=np.float32))
    eh = _bf16(embed)
    el = _bf16(embed - eh)
    ee = (embed.astype(np.float64) ** 2).sum(-1)
    sneg = -0.5 * ee
    r0 = _bf16(sneg.astype(np.float32))
    r1 = _bf16((sneg - r0).astype(np.float32))
    r2 = _bf16((sneg - r0 - r1).astype(np.float32))
    wpack = np.zeros((128, 5 * K + 128), dtype=BF)
    wpack[:, 0:2 * K] = eh.T.reshape(2, 128, K).transpose(1, 0, 2).reshape(128, 2 * K).astype(BF)
    wpack[:, 2 * K:4 * K] = el.T.reshape(2, 128, K).transpose(1, 0, 2).reshape(128, 2 * K).astype(BF)
    wpack[0, 4 * K:5 * K] = r0.astype(BF)
    wpack[1, 4 * K:5 * K] = r1.astype(BF)
    wpack[2, 4 * K:5 * K] = r2.astype(BF)
    wpack[0:3, 5 * K:5 * K + 128] = np.ones((3, 128), dtype=BF)
    identf = np.eye(128, dtype=np.float32)
    return [{"x": x[c], "wpack": wpack, "emb": embed, "identf": identf}
            for c in range(NCORES)]


def kernel(x: np.ndarray, embed: np.ndarray):
    assert np.asarray(x).shape == (B, S, D) and np.asarray(embed).shape == (K, D)
    in_maps = _prep_in_maps(x, embed)

    if "nc" not in _CACHE:
        _CACHE["nc"] = _build_program()
    nc = _CACHE["nc"]

    res = run_bass_kernel_spmd(nc, in_maps, list(range(NCORES))).results

    quantize = np.stack([res[c]["quant"] for c in range(NCORES)])
    embed_ind = np.stack([res[c]["ind"] for c in range(NCORES)]).view(np.int32)
    dist_sq = np.stack([res[c]["dist"] for c in range(NCORES)])
    num_expired = np.int32(0)
    return quantize, embed_ind, num_expired, dist_sq


# revision 21
# speedup vs baseline: 1.0177x; 1.0177x over previous
"""EuclideanCodebook (VQ) Trainium2 kernel.

Data-parallel over 8 NeuronCores: core i handles batch row i (8192 tokens,
64 tiles of 128). Per 128-token tile:
  score = x @ embed.T - 0.5*|e|^2   via bf16 3-term split matmuls
          (xh*eh + xh*el + xl*eh, fp32 PSUM accumulate; x split on-chip
          after a PE transpose, embed split on host) + a 3-row ones
          matmul folding -0.5*|e|^2, ~4e-4 abs accuracy -> zero argmax
          flips vs the fp32 reference.
  dist  = -2*score + |x|^2          (ACT Identity, per-partition bias)
  ind   = argmax(score)             (DVE max/max_index; ties -> first)
  quant = embed[ind]                (SWDGE indirect-DMA gather from HBM)
Tiles are processed in pairs (shared PE transpose + bf16 split,
software-pipelined one pair ahead so the ACT/DVE split never stalls the
matmul stream) and grouped into 8-tile supertiles for 1 MB-granularity
DMA; dist is written back per pair, quant per 4 tiles, indices in two
4 KB chunks. The ones/negee matmul operands span all 128 partitions
(rows 3..127 zero) so their LDWEIGHTS can pull ahead into the background
weight buffer instead of serializing on row-group 0.
Measured ~242 us/core on trn2 (PE-bound: product matmuls issue at the
216 ns/512-col hardware floor, ~84% PE occupancy; memory roofline for
the 49 MB/core of traffic is ~140 us).
"""
import sys
sys.path.insert(0, "/opt/trn_rl_repo")
from contextlib import ExitStack

import numpy as np
import ml_dtypes

import concourse.bass as bass
import concourse.bacc as bacc
import concourse.tile as tile
from concourse import mybir
from concourse.bass_utils import run_bass_kernel_spmd
from concourse.masks import make_identity

F32 = mybir.dt.float32
BF16 = mybir.dt.bfloat16
U32 = mybir.dt.uint32
ACT_T = mybir.ActivationFunctionType
BF = ml_dtypes.bfloat16

B, S, D, K = 8, 8192, 256, 1024
NCORES = 8
ST = 8                      # tiles per supertile
NS = S // (128 * ST)        # supertiles per core (8)
NT = ST * NS                # 64 token tiles per core


def _bf16(a):
    return np.asarray(a, np.float32).astype(BF).astype(np.float32)


def _build_program():
    nc = bacc.Bacc()
    x_d = nc.declare_dram_parameter("x", [S, D], F32, isOutput=False)
    wpack_d = nc.declare_dram_parameter("wpack", [128, 5 * K + 128], BF16, isOutput=False)
    identf_d = nc.declare_dram_parameter("identf", [128, 128], F32, isOutput=False)
    emb_d = nc.declare_dram_parameter("emb", [K, D], F32, isOutput=False)

    dist_d = nc.declare_dram_parameter("dist", [S, K], F32, isOutput=True)
    quant_d = nc.declare_dram_parameter("quant", [S, D], F32, isOutput=True)
    ind_d = nc.declare_dram_parameter("ind", [S], U32, isOutput=True)

    with tile.TileContext(nc) as tc, ExitStack() as ctx:
        sb = ctx.enter_context(tc.tile_pool(name="sb", bufs=1))
        sb2 = ctx.enter_context(tc.tile_pool(name="sb2", bufs=4))
        slab = ctx.enter_context(tc.tile_pool(name="slab", bufs=2))
        slabd = ctx.enter_context(tc.tile_pool(name="slabd", bufs=3))
        xslab = ctx.enter_context(tc.tile_pool(name="xslab", bufs=3))
        ps = ctx.enter_context(tc.tile_pool(name="ps", bufs=2, space="PSUM"))
        psd = ctx.enter_context(tc.tile_pool(name="psd", bufs=3, space="PSUM"))

        identf = sb.tile([128, 128], F32, tag="identf")
        nc.sync.dma_start(identf[:], identf_d[:])
        x_first = xslab.tile([128, ST, D], F32, tag="x")
        nc.sync.dma_start(
            x_first[:, 0:2, :], x_d[0:2 * 128, :].rearrange("(q p) d -> p q d", p=128))
        wp = sb.tile([128, 5 * K + 128], BF16, tag="wpack")
        nc.sync.dma_start(wp[:], wpack_d[:])
        nc.sync.dma_start(
            x_first[:, 2:ST, :],
            x_d[2 * 128:ST * 128, :].rearrange("(q p) d -> p q d", p=128))
        ones3 = wp[:, 5 * K:5 * K + 128]  # rows 3..127 are zero
        ind_cont = sb.tile([128, NT], U32, tag="indc")
        ind8_slab = sb.tile([128, NT * 8], U32, tag="ind8")
        indT = sb.tile([64, 128], U32, tag="indT")

        def ehT(c, ksl):
            return wp[:, c * K:(c + 1) * K][:, ksl]

        def elT(c, ksl):
            return wp[:, 2 * K + c * K:2 * K + (c + 1) * K][:, ksl]

        def negee(ksl):
            return wp[:, 4 * K:5 * K][:, ksl]  # rows 3..127 are zero

        def emit_ind_chunk(half):
            # tiles [half*32, half*32+32) -> ind_d[half*4096 : half*4096+4096]
            csl = slice(half * 32, half * 32 + 32)
            nc.vector.tensor_copy(
                ind_cont[:, csl],
                ind8_slab[:, half * 256:(half + 1) * 256].rearrange(
                    "p (t e) -> p t e", e=8)[:, :, 0])
            for bi in range(4):
                nc.vector.transpose(
                    indT[csl, bi * 32:(bi + 1) * 32],
                    ind_cont[bi * 32:(bi + 1) * 32, csl])
            nc.sync.dma_start(
                ind_d[half * 4096:(half + 1) * 4096].rearrange("(t p) -> t p", p=128),
                indT[csl, :])

        xsbs = {}

        def load_x(si):
            if si in xsbs or si >= NS:
                return
            t = xslab.tile([128, ST, D], F32, tag="x")
            nc.sync.dma_start(
                t[:],
                x_d[si * ST * 128:(si + 1) * ST * 128, :].rearrange(
                    "(q p) d -> p q d", p=128))
            xsbs[si] = t

        xsbs[0] = x_first
        load_x(1)

        def split_pair(si, q0):
            xs = xsbs[si]
            pt = ps.tile([128, 2 * D], F32, tag="pt")
            for j in range(2):
                for c in range(2):
                    nc.tensor.transpose(
                        pt[:, j * D + c * 128:j * D + (c + 1) * 128],
                        xs[:, q0 + j, c * 128:(c + 1) * 128], identf[:])
            xh = sb2.tile([128, 2 * D], BF16, tag="xh")
            nc.scalar.activation(xh[:], pt[:], ACT_T.Copy)
            xl = sb2.tile([128, 2 * D], BF16, tag="xl")
            nc.vector.tensor_sub(xl[:], pt[:], xh[:])
            return xh, xl

        cur = split_pair(0, 0)
        for s in range(NS):
            load_x(s + 1)
            x_sb = xsbs[s]
            dist_slab = slabd.tile([128, ST, K], F32, tag="dist")
            quant_slab = slab.tile([128, ST, D], F32, tag="quant")

            for qp in range(ST // 2):
                q0 = qp * 2
                xh, xl = cur
                if qp < ST // 2 - 1:
                    cur = split_pair(s, q0 + 2)
                elif s + 1 < NS:
                    cur = split_pair(s + 1, 0)

                for j in range(2):
                    q = q0 + j
                    t = s * ST + q
                    xsq = sb2.tile([128, 1], F32, tag="xsq")
                    sq_scr = sb2.tile([128, D], F32, tag="sqscr")
                    nc.scalar.activation(sq_scr[:], x_sb[:, q, :], ACT_T.Square,
                                         accum_out=xsq[:])

                    xhj = xh[:, j * D:(j + 1) * D]
                    xlj = xl[:, j * D:(j + 1) * D]
                    pd = psd.tile([128, K], F32, tag="pd")
                    for h in range(2):
                        ksl = slice(h * 512, (h + 1) * 512)
                        mms = [(ones3, negee(ksl)),
                               (xhj[:, 0:128], ehT(0, ksl)), (xhj[:, 128:256], ehT(1, ksl)),
                               (xhj[:, 0:128], elT(0, ksl)), (xhj[:, 128:256], elT(1, ksl)),
                               (xlj[:, 0:128], ehT(0, ksl)), (xlj[:, 128:256], ehT(1, ksl))]
                        for i, (lhs, rhs) in enumerate(mms):
                            nc.tensor.matmul(pd[:, h * 512:(h + 1) * 512], lhs, rhs,
                                             start=(i == 0), stop=(i == len(mms) - 1))

                    nc.scalar.activation(dist_slab[:, q, :], pd[:], ACT_T.Identity,
                                         bias=xsq[:], scale=-2.0)

                    max8 = sb2.tile([128, 8], F32, tag="max8")
                    nc.vector.max(max8[:], pd[:])
                    nc.vector.max_index(ind8_slab[:, t * 8:(t + 1) * 8], max8[:], pd[:])

                    nc.gpsimd.indirect_dma_start(
                        out=quant_slab[:, q, :], out_offset=None, in_=emb_d[:],
                        in_offset=bass.IndirectOffsetOnAxis(
                            ap=ind8_slab[:, t * 8:t * 8 + 1], axis=0))

                # dist out per pair (1 MB)
                base = s * ST * 128
                nc.sync.dma_start(
                    dist_d[base + q0 * 128:base + (q0 + 2) * 128, :].rearrange(
                        "(w p) k -> p w k", p=128),
                    dist_slab[:, q0:q0 + 2, :])
                if qp % 2 == 1:
                    h0 = q0 - 2
                    nc.sync.dma_start(
                        quant_d[base + h0 * 128:base + (q0 + 2) * 128, :].rearrange(
                            "(w p) d -> p w d", p=128),
                        quant_slab[:, h0:q0 + 2, :])
            if s == NS // 2 - 1:
                emit_ind_chunk(0)
        emit_ind_chunk(1)

    nc.finalize()
    return nc


_CACHE = {}


def _prep_in_maps(x, embed):
    x = np.ascontiguousarray(np.asarray(x, dtype=np.float32))
    embed = np.ascontiguousarray(np.asarray(embed, dtype=np.float32))
    eh = _bf16(embed)
    el = _bf16(embed - eh)
    ee = (embed.astype(np.float64) ** 2).sum(-1)
    sneg = -0.5 * ee
    r0 = _bf16(sneg.astype(np.float32))
    r1 = _bf16((sneg - r0).astype(np.float32))
    r2 = _bf16((sneg - r0 - r1).astype(np.float32))
    wpack = np.zeros((128, 5 * K + 128), dtype=BF)
    wpack[:, 0:2 * K] = eh.T.reshape(2, 128, K).transpose(1, 0, 2).reshape(128, 2 * K).astype(BF)
    wpack[:, 2 * K:4 * K] = el.T.reshape(2, 128, K).transpose(1, 0, 2).reshape(128, 2 * K).astype(BF)
    wpack[0, 4 * K:5 * K] = r0.astype(BF)
    wpack[1, 4 * K:5 * K] = r1.astype(BF)
    wpack[2, 4 * K:5 * K] = r2.astype(BF)
    wpack[0:3, 5 * K:5 * K + 128] = np.ones((3, 128), dtype=BF)
    identf = np.eye(128, dtype=np.float32)
    return [{"x": x[c], "wpack": wpack, "emb": embed, "identf": identf}
            for c in range(NCORES)]


def kernel(x: np.ndarray, embed: np.ndarray):
    assert np.asarray(x).shape == (B, S, D) and np.asarray(embed).shape == (K, D)
    in_maps = _prep_in_maps(x, embed)

    if "nc" not in _CACHE:
        _CACHE["nc"] = _build_program()
    nc = _CACHE["nc"]

    res = run_bass_kernel_spmd(nc, in_maps, list(range(NCORES))).results

    quantize = np.stack([res[c]["quant"] for c in range(NCORES)])
    embed_ind = np.stack([res[c]["ind"] for c in range(NCORES)]).view(np.int32)
    dist_sq = np.stack([res[c]["dist"] for c in range(NCORES)])
    num_expired = np.int32(0)
    return quantize, embed_ind, num_expired, dist_sq
